# revision 30
# baseline (speedup 1.0000x reference)
"""Bahdanau-style attention kernel for Trainium2 (Bass/Tile), 8-core SPMD.

Problem (per reference):
    enc_proj = einsum('bsh,oh->bso', enc_outputs, W_h)        # [B,S,H]
    dec_proj = einsum('bqh,oh->bqo', dec_hidden, W_s)         # [B,1,H]
    energy   = tanh(enc_proj + dec_proj)
    score    = einsum('bsh,h->bs', energy, v)                 # [B,S]
    weights  = softmax(score, -1)                             # [B,S]
    context  = einsum('bs,bsh->bh', weights, enc_outputs)[:,None,:]
    returns (context, weights)

B=64, S=2048, H=1024 fp32.  Dominant cost: the [B*S,H]@[H,H] projection
(275 GFLOP).  Sharding: data-parallel over batch, 8 batches per core,
weights replicated (per spec sharding_hint); no cross-core comms.

Per-core schedule (one batch = 16 s-blocks of 128 seq positions):
  - Main MM: out[s,o] += enc[s,h] * W_h[o,h].  Contraction h must sit on
    SBUF partitions for the PE, so the host pre-tiles enc into transposed
    [128h, 128s] stationary tiles (enc_t input) and pre-transposes W_h/W_s
    to [h,o] (moving operand, N=512 per PSUM bank).  float32r dtype runs
    the PE at 1 cycle/row (4x faster than float32).
  - dec_proj is broadcast to [128,H] once per batch (K=1 matmul) and added
    on VectorE (tensor_add from PSUM), keeping the PE stream pure matmul.
  - tanh on ScalarE; score via VectorE scalar_tensor_tensor
    (energy * v_bcast with accum_out row-sum) - NOT tensor_tensor_reduce,
    which crashes the device.
  - softmax without max-subtraction (scores are O(1); exp cannot overflow
    fp32), denominator accumulated at batch end; weights normalized then
    PE-transposed [128,16]->[16,128] for a contiguous DMA store.
  - context: unnormalized accumulation on VectorE (ping-ponged
    scalar_tensor_tensor: acc += enc_nat * exp_col), partition-reduced at
    batch end by a ones[128,1].T @ acc matmul, scaled by 1/denom during
    PSUM evacuation.  This keeps the PE at the 16-matmul/s-block floor;
    PE ~91% busy, DVE ~82%.
  - Work for s-block i that depends on the tanh->score->exp chain is
    emitted LAG=2 steps late so the PE's in-order FIFO never stalls on it.

PSUM budget (8 banks): main acc [128,1024] x3 bufs = 6, small
(denom/recip-bcast/weight-transpose/ctx-reduce) x2 = 2.

Measured on trn2 (8 cores): HW exec ~566 us/core, rel err ~1.7e-4
(float32r is TF32-class precision; plain fp32 runs 4x slower on the PE,
bf16 measured only ~4% faster with 16x the error).
"""

import os
import numpy as np

MM_MODE = os.environ.get("KERNEL_MM", "f32r")  # f32r | f16 | bf16 | f32
# 16-bit modes stream 2 cols/cycle on the PE, so the PE has slack there and
# the dec-add + context matmuls go back on it; for 4-byte modes the PE is
# saturated and both run on VectorE instead.
_CTX_DVE_DEFAULT = "1" if MM_MODE in ("f32r", "f32") else "0"
CTX_DVE = os.environ.get("KERNEL_CTX_DVE", _CTX_DVE_DEFAULT) == "1"

B, S, H = 64, 2048, 1024
NCORES = 8
B_LOC = B // NCORES            # 8 batches per core
P = 128
SB = S // P                    # 16 s-blocks per batch
KT = H // P                    # 8 contraction tiles
NB = 512                       # matmul free-dim (one fp32 PSUM bank)
NH = H // NB                   # 2 output halves
LAG = 2                        # deferred-emission distance for ctx MMs

_BUILT = {}


def _build(b_loc, s=S, h=H, mm="f32r", ctx_dve=CTX_DVE):
    from contextlib import ExitStack

    import concourse.bass as bass
    import concourse.bacc as bacc
    import concourse.tile as tile
    from concourse import mybir

    SB = s // P
    KT = h // P
    NH = h // NB
    S, H = s, h  # shadow module-level sizes inside the builder

    f32 = mybir.dt.float32
    mm_dt = {"f32r": mybir.dt.float32r, "f32": mybir.dt.float32,
             "bf16": mybir.dt.bfloat16, "f16": mybir.dt.float16}[mm]
    wide = mm in ("f32r", "f32")  # 4-byte operands: 1 col/cycle on the PE
    dec_pe = not wide             # 16-bit: add dec_proj via K=1 PE matmuls
    Act = mybir.ActivationFunctionType
    Alu = mybir.AluOpType

    nc = bacc.Bacc(trn_type="TRN2", target_bir_lowering=False)

    # [b, sb, p, kt*128]: enc_t[b, sb, p, kt*128 + sw] = enc[b, sb*128+sw, kt*128+p]
    enc_t = nc.dram_tensor("enc_t", [b_loc, SB, P, KT * P], mm_dt, kind="ExternalInput")
    enc = nc.dram_tensor("enc", [b_loc, S, H], mm_dt, kind="ExternalInput")
    # [p, kt*H]: w_ht[p, kt*H + o] = W_h[o, kt*128+p]
    w_ht = nc.dram_tensor("w_ht", [P, KT * H], mm_dt, kind="ExternalInput")
    w_st = nc.dram_tensor("w_st", [P, KT * H], mm_dt, kind="ExternalInput")
    # [p, kt*b_loc]: dec_t[p, kt*b_loc + b] = dec_hidden[b, 0, kt*128+p]
    dec_t = nc.dram_tensor("dec_t", [P, KT * b_loc], mm_dt, kind="ExternalInput")
    v_in = nc.dram_tensor("v_in", [H], f32, kind="ExternalInput")
    ones_in = nc.dram_tensor("ones_in", [1, P], mm_dt, kind="ExternalInput")
    ident_in = nc.dram_tensor("ident_in", [P, P], f32, kind="ExternalInput")
    ctx_out = nc.dram_tensor("ctx_out", [b_loc, H], f32, kind="ExternalOutput")
    wts_out = nc.dram_tensor("wts_out", [b_loc, S], f32, kind="ExternalOutput")

    with tile.TileContext(nc) as tc, ExitStack() as ctx:
        consts = ctx.enter_context(tc.tile_pool(name="consts", bufs=1))
        enc_t_pool = ctx.enter_context(tc.tile_pool(name="enc_t_pool", bufs=4))
        enc_pool = ctx.enter_context(tc.tile_pool(name="enc_pool", bufs=LAG + 3))
        en_pool = ctx.enter_context(tc.tile_pool(name="en_pool", bufs=2))
        scr_pool = ctx.enter_context(tc.tile_pool(name="scr_pool", bufs=2))
        col_pool = ctx.enter_context(tc.tile_pool(name="col_pool", bufs=LAG + 4))
        gather_pool = ctx.enter_context(tc.tile_pool(name="gather_pool", bufs=2))
        acc_pool = ctx.enter_context(tc.tile_pool(name="acc_pool", bufs=3))
        batch_pool = ctx.enter_context(tc.tile_pool(name="batch_pool", bufs=2))
        out_pool = ctx.enter_context(tc.tile_pool(name="out_pool", bufs=2))
        pmm = ctx.enter_context(tc.tile_pool(name="pmm", bufs=3 if ctx_dve else 2,
                                             space="PSUM"))
        if not ctx_dve:
            pctx = ctx.enter_context(tc.tile_pool(name="pctx", bufs=1, space="PSUM"))
        psmall = ctx.enter_context(tc.tile_pool(name="psmall", bufs=2, space="PSUM"))

        # ---- constants / weights ----
        # per-kt chunks so the first matmuls wait on ~512KB, not a full 4MB
        # transfer (w_st first: the dec preamble leads the PE FIFO)
        w_ht_sb = consts.tile([P, KT * H], mm_dt)
        w_st_sb = consts.tile([P, KT * H], mm_dt)
        dec_t_sb = consts.tile([P, KT * b_loc], mm_dt)
        nc.sync.dma_start(out=dec_t_sb, in_=dec_t[:, :])
        for kt in range(KT):
            sl = slice(kt * H, (kt + 1) * H)
            nc.sync.dma_start(out=w_st_sb[:, sl], in_=w_st[:, sl])
        prefetched = {}
        for kt in range(KT):
            sl = slice(kt * H, (kt + 1) * H)
            nc.sync.dma_start(out=w_ht_sb[:, sl], in_=w_ht[:, sl])

        v_sb = consts.tile([P, H], mm_dt if not wide else f32)
        v_ap = v_in[:]
        _v_bcast_ap = bass.AP(tensor=v_ap.tensor, offset=v_ap.offset,
                              ap=[[0, P]] + list(v_ap.ap))
        if wide:
            nc.sync.dma_start(out=v_sb, in_=_v_bcast_ap)
        else:
            # SWDGE casts f32 -> mm_dt during the broadcast
            nc.gpsimd.dma_start(out=v_sb, in_=_v_bcast_ap)

        ones_row = consts.tile([1, P], mm_dt)
        nc.sync.dma_start(out=ones_row, in_=ones_in[:, :])
        ones_col = consts.tile([P, 1], f32)
        nc.vector.memset(ones_col, 1.0)
        ones_row32 = consts.tile([1, P], f32)
        nc.vector.memset(ones_row32, 1.0)
        zeros_col = consts.tile([P, 1], f32)
        nc.vector.memset(zeros_col, 0.0)
        ident = consts.tile([P, P], f32)
        nc.sync.dma_start(out=ident, in_=ident_in[:, :])

        # ---- dec preamble: dec_proj = dec @ W_s^T ----
        dec_sb = consts.tile([b_loc, H], mm_dt)
        dec_rows = consts.tile([1, b_loc, H], mm_dt)

        def emit_dec_preamble():
            dec_ps = pmm.tile([b_loc, H], f32, tag="mm")
            for kt in range(KT):
                lhs = dec_t_sb[:, kt * b_loc:(kt + 1) * b_loc]
                for oh in range(NH):
                    nc.tensor.matmul(
                        dec_ps[:, oh * NB:(oh + 1) * NB],
                        lhsT=lhs,
                        rhs=w_st_sb[:, kt * H + oh * NB: kt * H + oh * NB + NB],
                        start=(kt == 0),
                        stop=(kt == KT - 1),
                    )
            nc.vector.tensor_copy(out=dec_sb, in_=dec_ps)
            # fold partitions 0..b_loc into free dim on partition 0
            nc.sync.dma_start(out=dec_rows[:, :, :], in_=dec_sb[:, :])

        # ---- main loop (software-pipelined emission) ----
        deferred = {}

        def defer(step, fn):
            deferred.setdefault(step, []).append(fn)

        state = {}

        def make_ctx_emitter(b, sb, st):
            def emit():
                enc_nat = st["enc_tiles"].pop(sb)
                if ctx_dve:
                    # acc_new = enc_nat * exp_col (+ acc_old)
                    exp_c32 = st["exp32_tiles"][sb]
                    acc_new = acc_pool.tile([P, H], f32, tag="acc",
                                            name=f"acc_{b}_{sb}")
                    if st["acc"] is None:
                        nc.vector.tensor_scalar_mul(acc_new, enc_nat.bitcast(f32),
                                                    exp_c32)
                    else:
                        nc.vector.scalar_tensor_tensor(
                            out=acc_new, in0=enc_nat.bitcast(f32),
                            scalar=exp_c32, in1=st["acc"],
                            op0=Alu.mult, op1=Alu.add)
                    st["acc"] = acc_new
                else:
                    lhs = st["exp_tiles"][sb]
                    for oh in range(NH):
                        nc.tensor.matmul(
                            st["ctx_ps"][:, oh * NB:(oh + 1) * NB],
                            lhsT=lhs,
                            rhs=enc_nat[:, oh * NB:(oh + 1) * NB],
                            start=(sb == 0),
                            stop=(sb == SB - 1),
                        )
            return emit

        def make_epilogue(b, st):
            def emit():
                exp_cols = st["exp_cols"]
                # denominator = sum over all partitions and s-blocks
                sums = col_pool.tile([P, 1], f32, tag="sums")
                nc.vector.reduce_sum(out=sums, in_=exp_cols,
                                     axis=mybir.AxisListType.X)
                den_ps = psmall.tile([1, 1], f32, tag="small")
                nc.tensor.matmul(den_ps, lhsT=sums, rhs=ones_col,
                                 start=True, stop=True)
                recip = batch_pool.tile([1, 1], f32, tag="recip")
                nc.vector.reciprocal(out=recip, in_=den_ps)
                # broadcast 1/denom to 128 partitions via K=1 matmul
                rb_ps = psmall.tile([P, 1], f32, tag="small")
                nc.tensor.matmul(rb_ps, lhsT=ones_row32, rhs=recip,
                                 start=True, stop=True)
                recip_bc = batch_pool.tile([P, 1], f32, tag="recip_bc")
                nc.vector.tensor_copy(out=recip_bc, in_=rb_ps)
                # normalized weights, transposed for contiguous store
                w_cols = batch_pool.tile([P, SB], f32, tag="w_cols")
                nc.vector.tensor_scalar_mul(w_cols, exp_cols, recip_bc)
                wt_ps = psmall.tile([SB, P], f32, tag="small")
                nc.tensor.transpose(wt_ps, w_cols, ident)
                wt_sb = out_pool.tile([SB, P], f32, tag="wt")
                nc.vector.tensor_copy(out=wt_sb, in_=wt_ps)
                nc.sync.dma_start(
                    out=wts_out[b].rearrange("(t p) -> t p", p=P), in_=wt_sb)
                # context, normalized during PSUM evacuation
                ctx_sb = out_pool.tile([1, H], f32, tag="ctxrow")
                if ctx_dve:
                    accT = st["acc"]
                    for oh in range(NH):
                        cr_ps = psmall.tile([1, NB], f32, tag="small",
                                            name=f"cr_ps_{b}_{oh}")
                        nc.tensor.matmul(
                            cr_ps,
                            lhsT=ones_col,
                            rhs=accT[:, oh * NB:(oh + 1) * NB],
                            start=True, stop=True)
                        nc.scalar.activation(out=ctx_sb[:, oh * NB:(oh + 1) * NB],
                                             in_=cr_ps, func=Act.Copy,
                                             bias=0.0, scale=recip)
                else:
                    nc.scalar.activation(out=ctx_sb, in_=st["ctx_ps"],
                                         func=Act.Copy, bias=0.0, scale=recip)
                nc.sync.dma_start(out=ctx_out[b:b + 1, :], in_=ctx_sb)
            return emit

        def emit_dec_bcast(b):
            # per-batch dec_proj broadcast [128, H] (PE K=1 matmul into a
            # psum slot, evacuated by DVE)
            db_ps = pmm.tile([P, H], f32, tag="mm", name=f"db_ps_{b}")
            for oh in range(NH):
                nc.tensor.matmul(
                    db_ps[:, oh * NB:(oh + 1) * NB],
                    lhsT=ones_row,
                    rhs=dec_rows[:, b, oh * NB:(oh + 1) * NB],
                    start=True, stop=True)
            dec_bc = gather_pool.tile([P, H], f32, tag="dec_bc",
                                      name=f"dec_bc_{b}")
            nc.vector.tensor_copy(out=dec_bc, in_=db_ps)
            state[b]["dec_bc"] = dec_bc

        emit_dec_preamble()

        G = b_loc * SB
        for g in range(G + LAG + 2):
            if g < G:
                b, sb = divmod(g, SB)
                if sb == 0:
                    state[b] = dict(
                        exp_cols=gather_pool.tile([P, SB], f32, tag="exp_cols",
                                                  name=f"exp_cols_b{b}"),
                        enc_tiles={},
                        exp_tiles={},
                        acc=None,
                    )
                    if not ctx_dve:
                        state[b]["ctx_ps"] = pctx.tile([1, H], f32, tag="ctx",
                                                       name=f"ctx_ps_b{b}")
                    if not dec_pe:
                        emit_dec_bcast(b)
                st = state[b]

                if g in prefetched:
                    enc_t_t, enc_nat = prefetched.pop(g)
                else:
                    enc_t_t = enc_t_pool.tile([P, KT * P], mm_dt, tag="enct",
                                              name=f"enct_{g}")
                    nc.sync.dma_start(out=enc_t_t, in_=enc_t[b, sb])
                    enc_nat = enc_pool.tile([P, H], mm_dt, tag="encn",
                                            name=f"encn_{g}")
                    nc.sync.dma_start(out=enc_nat,
                                      in_=enc[b, sb * P:(sb + 1) * P, :])
                st["enc_tiles"][sb] = enc_nat

                mm_ps = pmm.tile([P, H], f32, tag="mm", name=f"mm_ps_{g}")
                if dec_pe:
                    # open both bank groups with the dec_proj broadcast-add
                    for oh in range(NH):
                        nc.tensor.matmul(
                            mm_ps[:, oh * NB:(oh + 1) * NB],
                            lhsT=ones_row,
                            rhs=dec_rows[:, b, oh * NB:(oh + 1) * NB],
                            start=True, stop=False)
                # k-outer so each stationary enc_t tile serves both halves
                for kt in range(KT):
                    lhs = enc_t_t[:, kt * P:(kt + 1) * P]
                    for oh in range(NH):
                        nc.tensor.matmul(
                            mm_ps[:, oh * NB:(oh + 1) * NB],
                            lhsT=lhs,
                            rhs=w_ht_sb[:, kt * H + oh * NB: kt * H + oh * NB + NB],
                            start=(kt == 0 and not dec_pe),
                            stop=(kt == KT - 1),
                        )

                if dec_pe:
                    tanh_in = mm_ps
                else:
                    # dec_proj add on DVE (PSUM read), tanh on ACT
                    esum = scr_pool.tile([P, H], f32, tag="esum",
                                         name=f"esum_{g}")
                    nc.vector.tensor_add(esum, mm_ps, st["dec_bc"])
                    tanh_in = esum
                en_dt = mm_dt if not wide else f32
                energy = en_pool.tile([P, H], en_dt, tag="energy",
                                      name=f"energy_{g}")
                nc.scalar.activation(out=energy, in_=tanh_in, func=Act.Tanh,
                                     bias=zeros_col, scale=1.0)

                scr = scr_pool.tile([P, H], en_dt, tag="scr", name=f"scr_{g}")
                score_col = col_pool.tile([P, 1], f32, tag="score",
                                          name=f"score_{g}")
                nc.vector.scalar_tensor_tensor(
                    out=scr, in0=energy, scalar=1.0, in1=v_sb,
                    op0=Alu.mult, op1=Alu.mult, accum_out=score_col)

                exp_c = col_pool.tile([P, 1], f32, tag="expc", name=f"expc_{g}")
                nc.scalar.activation(out=exp_c, in_=score_col, func=Act.Exp,
                                     bias=zeros_col, scale=1.0)
                if ctx_dve:
                    st.setdefault("exp32_tiles", {})[sb] = exp_c
                else:
                    exp_r = col_pool.tile([P, 1], mm_dt, tag="expr",
                                          name=f"expr_{g}")
                    nc.vector.tensor_copy(out=exp_r, in_=exp_c)
                    st["exp_tiles"][sb] = exp_r
                # gather for the batch-end softmax denominator / store
                nc.vector.tensor_copy(out=st["exp_cols"][:, sb:sb + 1], in_=exp_c)

                defer(g + LAG, make_ctx_emitter(b, sb, st))
                if sb == SB - 1:
                    defer(g + LAG, make_epilogue(b, st))

            for fn in deferred.pop(g, []):
                fn()

        assert not deferred

    nc.compile()
    return nc


def _np_mm_dtype(mm):
    if mm == "bf16":
        import ml_dtypes
        return np.dtype(ml_dtypes.bfloat16)
    if mm == "f16":
        return np.dtype(np.float16)
    return np.dtype(np.float32)


def _prep_core_inputs(dec_hidden, enc_outputs, w_ht_host, w_st_host, v_host,
                      c, b_loc, s=S, h=H, mm=MM_MODE):
    np_dt = _np_mm_dtype(mm)
    sb_, kt_ = s // P, h // P
    lo, hi = c * b_loc, (c + 1) * b_loc
    enc = np.ascontiguousarray(enc_outputs[lo:hi], dtype=np_dt)
    # [b, sb, p, kt*128] with enc_t[b,sb,p,kt*128+sw] = enc[b, sb*128+sw, kt*128+p]
    e = enc.reshape(b_loc, sb_, P, kt_, P)
    enc_t = np.ascontiguousarray(e.transpose(0, 1, 4, 3, 2)).reshape(
        b_loc, sb_, P, kt_ * P)
    dec = np.asarray(dec_hidden[lo:hi, 0, :], dtype=np_dt)  # [b_loc, h]
    dec_t = np.ascontiguousarray(
        dec.T.reshape(kt_, P, b_loc).transpose(1, 0, 2)).reshape(P, kt_ * b_loc)
    return {
        "enc_t": enc_t,
        "enc": enc,
        "w_ht": w_ht_host,
        "w_st": w_st_host,
        "dec_t": dec_t,
        "v_in": v_host,
        "ones_in": np.ones((1, P), dtype=np_dt),
        "ident_in": np.eye(P, dtype=np.float32),
    }


def _swizzle_weight(w, h=H, mm=MM_MODE):
    # [o, h] -> [p, kt*h] with w_sw[p, kt*h + o] = w[o, kt*128+p]
    kt_ = h // P
    wt = np.asarray(w).astype(_np_mm_dtype(mm)).T  # [h, o]
    return np.ascontiguousarray(
        wt.reshape(kt_, P, w.shape[0]).transpose(1, 0, 2)).reshape(P, kt_ * w.shape[0])


def kernel(dec_hidden, enc_outputs, W_h, W_s, v, _trace=False):
    from concourse.bass_utils import run_bass_kernel_spmd

    assert enc_outputs.shape == (B, S, H)
    key = ("full", MM_MODE)
    if key not in _BUILT:
        _BUILT[key] = _build(B_LOC, mm=MM_MODE)
    nc = _BUILT[key]

    w_ht_host = _swizzle_weight(W_h)
    w_st_host = _swizzle_weight(W_s)
    v_host = np.ascontiguousarray(np.asarray(v, dtype=np.float32))

    in_maps = [
        _prep_core_inputs(dec_hidden, enc_outputs, w_ht_host, w_st_host,
                          v_host, c, B_LOC)
        for c in range(NCORES)
    ]
    res = run_bass_kernel_spmd(nc, in_maps, core_ids=list(range(NCORES)),
                               trace=_trace)
    kernel.last_results = res
    ctx = np.concatenate([r["ctx_out"] for r in res.results], axis=0)
    wts = np.concatenate([r["wts_out"] for r in res.results], axis=0)
    return (ctx.reshape(B, 1, H).astype(np.float32),
            wts.astype(np.float32))


# revision 31
# speedup vs baseline: 1.0060x; 1.0060x over previous
"""Bahdanau-style attention kernel for Trainium2 (Bass/Tile), 8-core SPMD.

Problem (per reference):
    enc_proj = einsum('bsh,oh->bso', enc_outputs, W_h)        # [B,S,H]
    dec_proj = einsum('bqh,oh->bqo', dec_hidden, W_s)         # [B,1,H]
    energy   = tanh(enc_proj + dec_proj)
    score    = einsum('bsh,h->bs', energy, v)                 # [B,S]
    weights  = softmax(score, -1)                             # [B,S]
    context  = einsum('bs,bsh->bh', weights, enc_outputs)[:,None,:]
    returns (context, weights)

B=64, S=2048, H=1024 fp32.  Dominant cost: the [B*S,H]@[H,H] projection
(275 GFLOP).  Sharding: data-parallel over batch, 8 batches per core,
weights replicated (per spec sharding_hint); no cross-core comms.

Per-core schedule (one batch = 16 s-blocks of 128 seq positions):
  - Main MM: out[s,o] += enc[s,h] * W_h[o,h].  Contraction h must sit on
    SBUF partitions for the PE, so the host pre-tiles enc into transposed
    [128h, 128s] stationary tiles (enc_t input) and pre-transposes W_h/W_s
    to [h,o] (moving operand, N=512 per PSUM bank).  float32r dtype runs
    the PE at 1 cycle/row (4x faster than float32).
  - dec_proj is broadcast to [128,H] once per batch (K=1 matmul) and added
    on VectorE (tensor_add from PSUM), keeping the PE stream pure matmul.
  - tanh on ScalarE; score via VectorE scalar_tensor_tensor
    (energy * v_bcast with accum_out row-sum) - NOT tensor_tensor_reduce,
    which crashes the device.
  - softmax without max-subtraction (scores are O(1); exp cannot overflow
    fp32), denominator accumulated at batch end; weights normalized then
    PE-transposed [128,16]->[16,128] for a contiguous DMA store.
  - context: unnormalized accumulation on VectorE (ping-ponged
    scalar_tensor_tensor: acc += enc_nat * exp_col), partition-reduced at
    batch end by a ones[128,1].T @ acc matmul, scaled by 1/denom during
    PSUM evacuation.  This keeps the PE at the 16-matmul/s-block floor;
    PE ~91% busy, DVE ~82%.
  - Work for s-block i that depends on the tanh->score->exp chain is
    emitted LAG=2 steps late so the PE's in-order FIFO never stalls on it.

PSUM budget (8 banks): main acc [128,1024] x3 bufs = 6, small
(denom/recip-bcast/weight-transpose/ctx-reduce) x2 = 2.

Measured on trn2 (8 cores): HW exec ~566 us/core, rel err ~1.7e-4
(float32r is TF32-class precision; plain fp32 runs 4x slower on the PE,
bf16 measured only ~4% faster with 16x the error).
"""

import os
import numpy as np

MM_MODE = os.environ.get("KERNEL_MM", "f32r")  # f32r | f16 | bf16 | f32
# 16-bit modes stream 2 cols/cycle on the PE, so the PE has slack there and
# the dec-add + context matmuls go back on it; for 4-byte modes the PE is
# saturated and both run on VectorE instead.
_CTX_DVE_DEFAULT = "1" if MM_MODE in ("f32r", "f32") else "0"
CTX_DVE = os.environ.get("KERNEL_CTX_DVE", _CTX_DVE_DEFAULT) == "1"

B, S, H = 64, 2048, 1024
NCORES = 8
B_LOC = B // NCORES            # 8 batches per core
P = 128
SB = S // P                    # 16 s-blocks per batch
KT = H // P                    # 8 contraction tiles
NB = 512                       # matmul free-dim (one fp32 PSUM bank)
NH = H // NB                   # 2 output halves
LAG = 2                        # deferred-emission distance for ctx MMs

_BUILT = {}


def _build(b_loc, s=S, h=H, mm="f32r", ctx_dve=CTX_DVE):
    from contextlib import ExitStack

    import concourse.bass as bass
    import concourse.bacc as bacc
    import concourse.tile as tile
    from concourse import mybir

    SB = s // P
    KT = h // P
    NH = h // NB
    S, H = s, h  # shadow module-level sizes inside the builder

    f32 = mybir.dt.float32
    mm_dt = {"f32r": mybir.dt.float32r, "f32": mybir.dt.float32,
             "bf16": mybir.dt.bfloat16, "f16": mybir.dt.float16}[mm]
    wide = mm in ("f32r", "f32")  # 4-byte operands: 1 col/cycle on the PE
    dec_pe = not wide             # 16-bit: add dec_proj via K=1 PE matmuls
    Act = mybir.ActivationFunctionType
    Alu = mybir.AluOpType

    nc = bacc.Bacc(trn_type="TRN2", target_bir_lowering=False)

    # [b, sb, p, kt*128]: enc_t[b, sb, p, kt*128 + sw] = enc[b, sb*128+sw, kt*128+p]
    enc_t = nc.dram_tensor("enc_t", [b_loc, SB, P, KT * P], mm_dt, kind="ExternalInput")
    enc = nc.dram_tensor("enc", [b_loc, S, H], mm_dt, kind="ExternalInput")
    # [p, kt*H]: w_ht[p, kt*H + o] = W_h[o, kt*128+p]
    w_ht = nc.dram_tensor("w_ht", [P, KT * H], mm_dt, kind="ExternalInput")
    w_st = nc.dram_tensor("w_st", [P, KT * H], mm_dt, kind="ExternalInput")
    # [p, kt*b_loc]: dec_t[p, kt*b_loc + b] = dec_hidden[b, 0, kt*128+p]
    dec_t = nc.dram_tensor("dec_t", [P, KT * b_loc], mm_dt, kind="ExternalInput")
    v_in = nc.dram_tensor("v_in", [H], f32, kind="ExternalInput")
    ones_in = nc.dram_tensor("ones_in", [1, P], mm_dt, kind="ExternalInput")
    ident_in = nc.dram_tensor("ident_in", [P, P], f32, kind="ExternalInput")
    ctx_out = nc.dram_tensor("ctx_out", [b_loc, H], f32, kind="ExternalOutput")
    wts_out = nc.dram_tensor("wts_out", [b_loc, S], f32, kind="ExternalOutput")

    with tile.TileContext(nc) as tc, ExitStack() as ctx:
        consts = ctx.enter_context(tc.tile_pool(name="consts", bufs=1))
        enc_t_pool = ctx.enter_context(tc.tile_pool(name="enc_t_pool", bufs=4))
        enc_pool = ctx.enter_context(tc.tile_pool(name="enc_pool", bufs=LAG + 3))
        en_pool = ctx.enter_context(tc.tile_pool(name="en_pool", bufs=2))
        scr_pool = ctx.enter_context(tc.tile_pool(name="scr_pool", bufs=2))
        col_pool = ctx.enter_context(tc.tile_pool(name="col_pool", bufs=LAG + 4))
        gather_pool = ctx.enter_context(tc.tile_pool(name="gather_pool", bufs=2))
        acc_pool = ctx.enter_context(tc.tile_pool(name="acc_pool", bufs=3))
        batch_pool = ctx.enter_context(tc.tile_pool(name="batch_pool", bufs=2))
        out_pool = ctx.enter_context(tc.tile_pool(name="out_pool", bufs=2))
        pmm = ctx.enter_context(tc.tile_pool(name="pmm", bufs=3 if ctx_dve else 2,
                                             space="PSUM"))
        if not ctx_dve:
            pctx = ctx.enter_context(tc.tile_pool(name="pctx", bufs=1, space="PSUM"))
        psmall = ctx.enter_context(tc.tile_pool(name="psmall", bufs=2, space="PSUM"))

        # ---- constants / weights ----
        # per-kt chunks so the first matmuls wait on ~512KB, not a full 4MB
        # transfer (w_st first: the dec preamble leads the PE FIFO)
        w_ht_sb = consts.tile([P, KT * H], mm_dt)
        w_st_sb = consts.tile([P, KT * H], mm_dt)
        dec_t_sb = consts.tile([P, KT * b_loc], mm_dt)
        nc.sync.dma_start(out=dec_t_sb, in_=dec_t[:, :])
        for kt in range(KT):
            sl = slice(kt * H, (kt + 1) * H)
            nc.sync.dma_start(out=w_st_sb[:, sl], in_=w_st[:, sl])
        prefetched = {}
        for kt in range(KT):
            sl = slice(kt * H, (kt + 1) * H)
            nc.sync.dma_start(out=w_ht_sb[:, sl], in_=w_ht[:, sl])

        v_sb = consts.tile([P, H], mm_dt if not wide else f32)
        v_ap = v_in[:]
        _v_bcast_ap = bass.AP(tensor=v_ap.tensor, offset=v_ap.offset,
                              ap=[[0, P]] + list(v_ap.ap))
        if wide:
            nc.sync.dma_start(out=v_sb, in_=_v_bcast_ap)
        else:
            # SWDGE casts f32 -> mm_dt during the broadcast
            nc.gpsimd.dma_start(out=v_sb, in_=_v_bcast_ap)

        ones_row = consts.tile([1, P], mm_dt)
        nc.sync.dma_start(out=ones_row, in_=ones_in[:, :])
        ones_col = consts.tile([P, 1], f32)
        nc.vector.memset(ones_col, 1.0)
        ones_row32 = consts.tile([1, P], f32)
        nc.vector.memset(ones_row32, 1.0)
        zeros_col = consts.tile([P, 1], f32)
        nc.vector.memset(zeros_col, 0.0)
        ident = consts.tile([P, P], f32)
        nc.sync.dma_start(out=ident, in_=ident_in[:, :])

        # ---- dec preamble: dec_proj = dec @ W_s^T ----
        dec_sb = consts.tile([b_loc, H], mm_dt)
        dec_rows = consts.tile([1, b_loc, H], mm_dt)

        def emit_dec_preamble():
            dec_ps = pmm.tile([b_loc, H], f32, tag="mm")
            for kt in range(KT):
                lhs = dec_t_sb[:, kt * b_loc:(kt + 1) * b_loc]
                for oh in range(NH):
                    nc.tensor.matmul(
                        dec_ps[:, oh * NB:(oh + 1) * NB],
                        lhsT=lhs,
                        rhs=w_st_sb[:, kt * H + oh * NB: kt * H + oh * NB + NB],
                        start=(kt == 0),
                        stop=(kt == KT - 1),
                    )
            nc.vector.tensor_copy(out=dec_sb, in_=dec_ps)
            # fold partitions 0..b_loc into free dim on partition 0
            nc.sync.dma_start(out=dec_rows[:, :, :], in_=dec_sb[:, :])

        # ---- main loop (software-pipelined emission) ----
        deferred = {}

        def defer(step, fn):
            deferred.setdefault(step, []).append(fn)

        state = {}

        def make_ctx_emitter(b, sb, st):
            def emit():
                enc_nat = st["enc_tiles"].pop(sb)
                if ctx_dve:
                    # acc_new = enc_nat * exp_col (+ acc_old)
                    exp_c32 = st["exp32_tiles"][sb]
                    acc_new = acc_pool.tile([P, H], f32, tag="acc",
                                            name=f"acc_{b}_{sb}")
                    if st["acc"] is None:
                        nc.vector.tensor_scalar_mul(acc_new, enc_nat.bitcast(f32),
                                                    exp_c32)
                    else:
                        nc.vector.scalar_tensor_tensor(
                            out=acc_new, in0=enc_nat.bitcast(f32),
                            scalar=exp_c32, in1=st["acc"],
                            op0=Alu.mult, op1=Alu.add)
                    st["acc"] = acc_new
                else:
                    lhs = st["exp_tiles"][sb]
                    for oh in range(NH):
                        nc.tensor.matmul(
                            st["ctx_ps"][:, oh * NB:(oh + 1) * NB],
                            lhsT=lhs,
                            rhs=enc_nat[:, oh * NB:(oh + 1) * NB],
                            start=(sb == 0),
                            stop=(sb == SB - 1),
                        )
            return emit

        def make_epilogue(b, st):
            def emit():
                exp_cols = st["exp_cols"]
                # denominator = sum over all partitions and s-blocks
                sums = col_pool.tile([P, 1], f32, tag="sums")
                nc.vector.reduce_sum(out=sums, in_=exp_cols,
                                     axis=mybir.AxisListType.X)
                den_ps = psmall.tile([1, 1], f32, tag="small")
                nc.tensor.matmul(den_ps, lhsT=sums, rhs=ones_col,
                                 start=True, stop=True)
                recip = batch_pool.tile([1, 1], f32, tag="recip")
                nc.vector.reciprocal(out=recip, in_=den_ps)
                # broadcast 1/denom to 128 partitions via K=1 matmul
                rb_ps = psmall.tile([P, 1], f32, tag="small")
                nc.tensor.matmul(rb_ps, lhsT=ones_row32, rhs=recip,
                                 start=True, stop=True)
                recip_bc = batch_pool.tile([P, 1], f32, tag="recip_bc")
                nc.vector.tensor_copy(out=recip_bc, in_=rb_ps)
                # normalized weights, transposed for contiguous store
                w_cols = batch_pool.tile([P, SB], f32, tag="w_cols")
                nc.vector.tensor_scalar_mul(w_cols, exp_cols, recip_bc)
                wt_ps = psmall.tile([SB, P], f32, tag="small")
                nc.tensor.transpose(wt_ps, w_cols, ident)
                wt_sb = out_pool.tile([SB, P], f32, tag="wt")
                nc.vector.tensor_copy(out=wt_sb, in_=wt_ps)
                nc.sync.dma_start(
                    out=wts_out[b].rearrange("(t p) -> t p", p=P), in_=wt_sb)
                # context, normalized during PSUM evacuation
                ctx_sb = out_pool.tile([1, H], f32, tag="ctxrow")
                if ctx_dve:
                    accT = st["acc"]
                    for oh in range(NH):
                        cr_ps = psmall.tile([1, NB], f32, tag="small",
                                            name=f"cr_ps_{b}_{oh}")
                        nc.tensor.matmul(
                            cr_ps,
                            lhsT=ones_col,
                            rhs=accT[:, oh * NB:(oh + 1) * NB],
                            start=True, stop=True)
                        nc.scalar.activation(out=ctx_sb[:, oh * NB:(oh + 1) * NB],
                                             in_=cr_ps, func=Act.Copy,
                                             bias=0.0, scale=recip)
                else:
                    nc.scalar.activation(out=ctx_sb, in_=st["ctx_ps"],
                                         func=Act.Copy, bias=0.0, scale=recip)
                nc.sync.dma_start(out=ctx_out[b:b + 1, :], in_=ctx_sb)
            return emit

        def emit_dec_bcast(b):
            # per-batch dec_proj broadcast [128, H] (PE K=1 matmul into a
            # psum slot, evacuated by DVE)
            db_ps = pmm.tile([P, H], f32, tag="mm", name=f"db_ps_{b}")
            for oh in range(NH):
                nc.tensor.matmul(
                    db_ps[:, oh * NB:(oh + 1) * NB],
                    lhsT=ones_row,
                    rhs=dec_rows[:, b, oh * NB:(oh + 1) * NB],
                    start=True, stop=True)
            dec_bc = gather_pool.tile([P, H], f32, tag="dec_bc",
                                      name=f"dec_bc_{b}")
            nc.vector.tensor_copy(out=dec_bc, in_=db_ps)
            state[b]["dec_bc"] = dec_bc

        emit_dec_preamble()

        G = b_loc * SB
        for g in range(G + LAG + 2):
            if g < G:
                b, sb = divmod(g, SB)
                if sb == 0:
                    state[b] = dict(
                        exp_cols=gather_pool.tile([P, SB], f32, tag="exp_cols",
                                                  name=f"exp_cols_b{b}"),
                        enc_tiles={},
                        exp_tiles={},
                        acc=None,
                    )
                    if not ctx_dve:
                        state[b]["ctx_ps"] = pctx.tile([1, H], f32, tag="ctx",
                                                       name=f"ctx_ps_b{b}")
                    if not dec_pe:
                        emit_dec_bcast(b)
                st = state[b]

                if g in prefetched:
                    enc_t_t, enc_nat = prefetched.pop(g)
                else:
                    enc_t_t = enc_t_pool.tile([P, KT * P], mm_dt, tag="enct",
                                              name=f"enct_{g}")
                    nc.sync.dma_start(out=enc_t_t, in_=enc_t[b, sb])
                    enc_nat = enc_pool.tile([P, H], mm_dt, tag="encn",
                                            name=f"encn_{g}")
                    nc.sync.dma_start(out=enc_nat,
                                      in_=enc[b, sb * P:(sb + 1) * P, :])
                st["enc_tiles"][sb] = enc_nat

                mm_ps = pmm.tile([P, H], f32, tag="mm", name=f"mm_ps_{g}")
                if dec_pe:
                    # open both bank groups with the dec_proj broadcast-add
                    for oh in range(NH):
                        nc.tensor.matmul(
                            mm_ps[:, oh * NB:(oh + 1) * NB],
                            lhsT=ones_row,
                            rhs=dec_rows[:, b, oh * NB:(oh + 1) * NB],
                            start=True, stop=False)
                # k-outer so each stationary enc_t tile serves both halves
                for kt in range(KT):
                    lhs = enc_t_t[:, kt * P:(kt + 1) * P]
                    for oh in range(NH):
                        nc.tensor.matmul(
                            mm_ps[:, oh * NB:(oh + 1) * NB],
                            lhsT=lhs,
                            rhs=w_ht_sb[:, kt * H + oh * NB: kt * H + oh * NB + NB],
                            start=(kt == 0 and not dec_pe),
                            stop=(kt == KT - 1),
                        )

                if dec_pe:
                    tanh_in = mm_ps
                else:
                    # dec_proj add on DVE (PSUM read), tanh on ACT
                    esum = scr_pool.tile([P, H], f32, tag="esum",
                                         name=f"esum_{g}")
                    nc.vector.tensor_add(esum, mm_ps, st["dec_bc"])
                    tanh_in = esum
                en_dt = mm_dt if not wide else f32
                energy = en_pool.tile([P, H], en_dt, tag="energy",
                                      name=f"energy_{g}")
                nc.scalar.activation(out=energy, in_=tanh_in, func=Act.Tanh,
                                     bias=zeros_col, scale=1.0)

                scr = scr_pool.tile([P, H], en_dt, tag="scr", name=f"scr_{g}")
                score_col = col_pool.tile([P, 1], f32, tag="score",
                                          name=f"score_{g}")
                nc.vector.scalar_tensor_tensor(
                    out=scr, in0=energy, scalar=1.0, in1=v_sb,
                    op0=Alu.mult, op1=Alu.mult, accum_out=score_col)

                exp_c = col_pool.tile([P, 1], f32, tag="expc", name=f"expc_{g}")
                nc.scalar.activation(out=exp_c, in_=score_col, func=Act.Exp,
                                     bias=zeros_col, scale=1.0)
                if ctx_dve:
                    st.setdefault("exp32_tiles", {})[sb] = exp_c
                else:
                    exp_r = col_pool.tile([P, 1], mm_dt, tag="expr",
                                          name=f"expr_{g}")
                    nc.vector.tensor_copy(out=exp_r, in_=exp_c)
                    st["exp_tiles"][sb] = exp_r
                # gather for the batch-end softmax denominator / store
                nc.vector.tensor_copy(out=st["exp_cols"][:, sb:sb + 1], in_=exp_c)

                # tighter lag for the final batch's last blocks: the tail
                # chain is fully serial, so emit dependents ASAP there
                lag = 1 if (b == b_loc - 1 and sb >= SB - 2) else LAG
                defer(g + lag, make_ctx_emitter(b, sb, st))
                if sb == SB - 1:
                    defer(g + lag, make_epilogue(b, st))

            for fn in deferred.pop(g, []):
                fn()

        assert not deferred

    nc.compile()
    return nc


def _np_mm_dtype(mm):
    if mm == "bf16":
        import ml_dtypes
        return np.dtype(ml_dtypes.bfloat16)
    if mm == "f16":
        return np.dtype(np.float16)
    return np.dtype(np.float32)


def _prep_core_inputs(dec_hidden, enc_outputs, w_ht_host, w_st_host, v_host,
                      c, b_loc, s=S, h=H, mm=MM_MODE):
    np_dt = _np_mm_dtype(mm)
    sb_, kt_ = s // P, h // P
    lo, hi = c * b_loc, (c + 1) * b_loc
    enc = np.ascontiguousarray(enc_outputs[lo:hi], dtype=np_dt)
    # [b, sb, p, kt*128] with enc_t[b,sb,p,kt*128+sw] = enc[b, sb*128+sw, kt*128+p]
    e = enc.reshape(b_loc, sb_, P, kt_, P)
    enc_t = np.ascontiguousarray(e.transpose(0, 1, 4, 3, 2)).reshape(
        b_loc, sb_, P, kt_ * P)
    dec = np.asarray(dec_hidden[lo:hi, 0, :], dtype=np_dt)  # [b_loc, h]
    dec_t = np.ascontiguousarray(
        dec.T.reshape(kt_, P, b_loc).transpose(1, 0, 2)).reshape(P, kt_ * b_loc)
    return {
        "enc_t": enc_t,
        "enc": enc,
        "w_ht": w_ht_host,
        "w_st": w_st_host,
        "dec_t": dec_t,
        "v_in": v_host,
        "ones_in": np.ones((1, P), dtype=np_dt),
        "ident_in": np.eye(P, dtype=np.float32),
    }


def _swizzle_weight(w, h=H, mm=MM_MODE):
    # [o, h] -> [p, kt*h] with w_sw[p, kt*h + o] = w[o, kt*128+p]
    kt_ = h // P
    wt = np.asarray(w).astype(_np_mm_dtype(mm)).T  # [h, o]
    return np.ascontiguousarray(
        wt.reshape(kt_, P, w.shape[0]).transpose(1, 0, 2)).reshape(P, kt_ * w.shape[0])


def kernel(dec_hidden, enc_outputs, W_h, W_s, v, _trace=False):
    from concourse.bass_utils import run_bass_kernel_spmd

    assert enc_outputs.shape == (B, S, H)
    key = ("full", MM_MODE)
    if key not in _BUILT:
        _BUILT[key] = _build(B_LOC, mm=MM_MODE)
    nc = _BUILT[key]

    w_ht_host = _swizzle_weight(W_h)
    w_st_host = _swizzle_weight(W_s)
    v_host = np.ascontiguousarray(np.asarray(v, dtype=np.float32))

    in_maps = [
        _prep_core_inputs(dec_hidden, enc_outputs, w_ht_host, w_st_host,
                          v_host, c, B_LOC)
        for c in range(NCORES)
    ]
    res = run_bass_kernel_spmd(nc, in_maps, core_ids=list(range(NCORES)),
                               trace=_trace)
    kernel.last_results = res
    ctx = np.concatenate([r["ctx_out"] for r in res.results], axis=0)
    wts = np.concatenate([r["wts_out"] for r in res.results], axis=0)
    return (ctx.reshape(B, 1, H).astype(np.float32),
            wts.astype(np.float32))


# revision 33
# speedup vs baseline: 1.1003x; 1.0938x over previous
"""Bahdanau-style attention kernel for Trainium2 (Bass/Tile), 8-core SPMD.

Problem (per reference):
    enc_proj = einsum('bsh,oh->bso', enc_outputs, W_h)        # [B,S,H]
    dec_proj = einsum('bqh,oh->bqo', dec_hidden, W_s)         # [B,1,H]
    energy   = tanh(enc_proj + dec_proj)
    score    = einsum('bsh,h->bs', energy, v)                 # [B,S]
    weights  = softmax(score, -1)                             # [B,S]
    context  = einsum('bs,bsh->bh', weights, enc_outputs)[:,None,:]
    returns (context, weights)

B=64, S=2048, H=1024 fp32.  Dominant cost: the [B*S,H]@[H,H] projection
(275 GFLOP).  Sharding: data-parallel over batch, 8 batches per core,
weights replicated (per spec sharding_hint); no cross-core comms.

Per-core schedule (one batch = 16 s-blocks of 128 seq positions):
  - Main MM: out[s,o] += enc[s,h] * W_h[o,h].  Contraction h must sit on
    SBUF partitions for the PE, so the host pre-tiles enc into transposed
    [128h, 128s] stationary tiles (enc_t input) and pre-transposes W_h/W_s
    to [h,o] (moving operand, N=512 per PSUM bank).  float32r dtype runs
    the PE at 1 cycle/row (4x faster than float32).
  - dec_proj is broadcast to [128,H] once per batch (K=1 matmul) and added
    on VectorE (tensor_add from PSUM), keeping the PE stream pure matmul.
  - tanh on ScalarE; score via VectorE scalar_tensor_tensor
    (energy * v_bcast with accum_out row-sum) - NOT tensor_tensor_reduce,
    which crashes the device.
  - softmax without max-subtraction (scores are O(1); exp cannot overflow
    fp32), denominator accumulated at batch end; weights normalized then
    PE-transposed [128,16]->[16,128] for a contiguous DMA store.
  - context: unnormalized accumulation on VectorE (ping-ponged
    scalar_tensor_tensor: acc += enc_nat * exp_col), partition-reduced at
    batch end by a ones[128,1].T @ acc matmul, scaled by 1/denom during
    PSUM evacuation.  This keeps the PE at the 16-matmul/s-block floor;
    PE ~91% busy, DVE ~82%.
  - Work for s-block i that depends on the tanh->score->exp chain is
    emitted LAG=2 steps late so the PE's in-order FIFO never stalls on it.

PSUM budget (8 banks): main acc [128,1024] x3 bufs = 6, small
(denom/recip-bcast/weight-transpose/ctx-reduce) x2 = 2.

Measured on trn2 (8 cores): HW exec ~563 us/core, rel err ~1.7e-4
(float32r is TF32-class precision; plain fp32 runs 4x slower on the PE,
bf16 measured only ~4% faster with 16x the error).
"""

import os
import numpy as np

MM_MODE = os.environ.get("KERNEL_MM", "f32r")  # f32r | f16 | bf16 | f32
# 16-bit modes stream 2 cols/cycle on the PE, so the PE has slack there and
# the dec-add + context matmuls go back on it; for 4-byte modes the PE is
# saturated and both run on VectorE instead.
_CTX_DVE_DEFAULT = "1" if MM_MODE in ("f32r", "f32") else "0"
CTX_DVE = os.environ.get("KERNEL_CTX_DVE", _CTX_DVE_DEFAULT) == "1"

B, S, H = 64, 2048, 1024
NCORES = 8
B_LOC = B // NCORES            # 8 batches per core
P = 128
SB = S // P                    # 16 s-blocks per batch
KT = H // P                    # 8 contraction tiles
NB = 512                       # matmul free-dim (one fp32 PSUM bank)
NH = H // NB                   # 2 output halves
LAG = 2                        # deferred-emission distance for ctx MMs

_BUILT = {}


def _build(b_loc, s=S, h=H, mm="f32r", ctx_dve=CTX_DVE):
    import os
    from contextlib import ExitStack

    import concourse.bass as bass
    import concourse.bacc as bacc
    import concourse.tile as tile
    from concourse import mybir

    SB = s // P
    KT = h // P
    NH = h // NB
    S, H = s, h  # shadow module-level sizes inside the builder

    f32 = mybir.dt.float32
    mm_dt = {"f32r": mybir.dt.float32r, "f32": mybir.dt.float32,
             "bf16": mybir.dt.bfloat16, "f16": mybir.dt.float16}[mm]
    wide = mm in ("f32r", "f32")  # 4-byte operands: 1 col/cycle on the PE
    _dp = os.environ.get("KERNEL_DEC_PE")
    dec_pe = (_dp == "1") if _dp is not None else not wide
    Act = mybir.ActivationFunctionType
    Alu = mybir.AluOpType

    nc = bacc.Bacc(trn_type="TRN2", target_bir_lowering=False)

    # [b, sb, p, kt*128]: enc_t[b, sb, p, kt*128 + sw] = enc[b, sb*128+sw, kt*128+p]
    enc_t = nc.dram_tensor("enc_t", [b_loc, SB, P, KT * P], mm_dt, kind="ExternalInput")
    enc = nc.dram_tensor("enc", [b_loc, S, H], mm_dt, kind="ExternalInput")
    # [p, kt*H]: w_ht[p, kt*H + o] = W_h[o, kt*128+p]
    w_ht = nc.dram_tensor("w_ht", [P, KT * H], mm_dt, kind="ExternalInput")
    w_st = nc.dram_tensor("w_st", [P, KT * H], mm_dt, kind="ExternalInput")
    # [p, kt*b_loc]: dec_t[p, kt*b_loc + b] = dec_hidden[b, 0, kt*128+p]
    dec_t = nc.dram_tensor("dec_t", [P, KT * b_loc], mm_dt, kind="ExternalInput")
    v_in = nc.dram_tensor("v_in", [H], f32, kind="ExternalInput")
    ones_in = nc.dram_tensor("ones_in", [1, P], mm_dt, kind="ExternalInput")
    ident_in = nc.dram_tensor("ident_in", [P, P], f32, kind="ExternalInput")
    ctx_out = nc.dram_tensor("ctx_out", [b_loc, H], f32, kind="ExternalOutput")
    wts_out = nc.dram_tensor("wts_out", [b_loc, S], f32, kind="ExternalOutput")

    with tile.TileContext(nc) as tc, ExitStack() as ctx:
        consts = ctx.enter_context(tc.tile_pool(name="consts", bufs=1))
        enc_t_pool = ctx.enter_context(tc.tile_pool(name="enc_t_pool", bufs=4))
        enc_pool = ctx.enter_context(tc.tile_pool(name="enc_pool", bufs=LAG + 3))
        en_pool = ctx.enter_context(tc.tile_pool(name="en_pool", bufs=2))
        scr_pool = ctx.enter_context(tc.tile_pool(name="scr_pool", bufs=2))
        col_pool = ctx.enter_context(tc.tile_pool(name="col_pool", bufs=LAG + 4))
        gather_pool = ctx.enter_context(tc.tile_pool(name="gather_pool", bufs=2))
        acc_pool = ctx.enter_context(tc.tile_pool(name="acc_pool", bufs=3))
        batch_pool = ctx.enter_context(tc.tile_pool(name="batch_pool", bufs=2))
        out_pool = ctx.enter_context(tc.tile_pool(name="out_pool", bufs=2))
        pmm = ctx.enter_context(tc.tile_pool(name="pmm", bufs=3 if ctx_dve else 2,
                                             space="PSUM"))
        if not ctx_dve:
            pctx = ctx.enter_context(tc.tile_pool(name="pctx", bufs=1, space="PSUM"))
        psmall = ctx.enter_context(tc.tile_pool(name="psmall", bufs=2, space="PSUM"))

        # ---- constants / weights ----
        # per-kt chunks so the first matmuls wait on ~512KB, not a full 4MB
        # transfer (w_st first: the dec preamble leads the PE FIFO)
        w_ht_sb = consts.tile([P, KT * H], mm_dt)
        w_st_sb = consts.tile([P, KT * H], mm_dt)
        dec_t_sb = consts.tile([P, KT * b_loc], mm_dt)
        nc.sync.dma_start(out=dec_t_sb, in_=dec_t[:, :])
        for kt in range(KT):
            sl = slice(kt * H, (kt + 1) * H)
            nc.sync.dma_start(out=w_st_sb[:, sl], in_=w_st[:, sl])
        prefetched = {}
        for kt in range(KT):
            sl = slice(kt * H, (kt + 1) * H)
            nc.sync.dma_start(out=w_ht_sb[:, sl], in_=w_ht[:, sl])

        v_sb = consts.tile([P, H], mm_dt if not wide else f32)
        v_ap = v_in[:]
        _v_bcast_ap = bass.AP(tensor=v_ap.tensor, offset=v_ap.offset,
                              ap=[[0, P]] + list(v_ap.ap))
        if wide:
            nc.sync.dma_start(out=v_sb, in_=_v_bcast_ap)
        else:
            # SWDGE casts f32 -> mm_dt during the broadcast
            nc.gpsimd.dma_start(out=v_sb, in_=_v_bcast_ap)

        ones_row = consts.tile([1, P], mm_dt)
        nc.sync.dma_start(out=ones_row, in_=ones_in[:, :])
        ones_col = consts.tile([P, 1], f32)
        nc.vector.memset(ones_col, 1.0)
        ones_row32 = consts.tile([1, P], f32)
        nc.vector.memset(ones_row32, 1.0)
        zeros_col = consts.tile([P, 1], f32)
        nc.vector.memset(zeros_col, 0.0)
        ident = consts.tile([P, P], f32)
        nc.sync.dma_start(out=ident, in_=ident_in[:, :])

        # ---- dec preamble: dec_proj = dec @ W_s^T ----
        dec_sb = consts.tile([b_loc, H], mm_dt)
        dec_rows = consts.tile([1, b_loc, H], mm_dt)

        def emit_dec_preamble():
            dec_ps = pmm.tile([b_loc, H], f32, tag="mm")
            for kt in range(KT):
                lhs = dec_t_sb[:, kt * b_loc:(kt + 1) * b_loc]
                for oh in range(NH):
                    nc.tensor.matmul(
                        dec_ps[:, oh * NB:(oh + 1) * NB],
                        lhsT=lhs,
                        rhs=w_st_sb[:, kt * H + oh * NB: kt * H + oh * NB + NB],
                        start=(kt == 0),
                        stop=(kt == KT - 1),
                    )
            nc.vector.tensor_copy(out=dec_sb, in_=dec_ps)
            # fold partitions 0..b_loc into free dim on partition 0
            nc.sync.dma_start(out=dec_rows[:, :, :], in_=dec_sb[:, :])

        # ---- main loop (software-pipelined emission) ----
        deferred = {}

        def defer(step, fn):
            deferred.setdefault(step, []).append(fn)

        state = {}

        def make_ctx_emitter(b, sb, st):
            def emit():
                enc_nat = st["enc_tiles"].pop(sb)
                if ctx_dve:
                    # acc_new = enc_nat * exp_col (+ acc_old)
                    exp_c32 = st["exp32_tiles"][sb]
                    acc_new = acc_pool.tile([P, H], f32, tag="acc",
                                            name=f"acc_{b}_{sb}")
                    enc_in = enc_nat.bitcast(f32) if wide else enc_nat
                    if st["acc"] is None:
                        nc.vector.tensor_scalar_mul(acc_new, enc_in, exp_c32)
                    else:
                        nc.vector.scalar_tensor_tensor(
                            out=acc_new, in0=enc_in,
                            scalar=exp_c32, in1=st["acc"],
                            op0=Alu.mult, op1=Alu.add)
                    st["acc"] = acc_new
                else:
                    lhs = st["exp_tiles"][sb]
                    for oh in range(NH):
                        nc.tensor.matmul(
                            st["ctx_ps"][:, oh * NB:(oh + 1) * NB],
                            lhsT=lhs,
                            rhs=enc_nat[:, oh * NB:(oh + 1) * NB],
                            start=(sb == 0),
                            stop=(sb == SB - 1),
                        )
            return emit

        def make_epilogue(b, st):
            def emit():
                exp_cols = st["exp_cols"]
                # denominator = sum over all partitions and s-blocks
                sums = col_pool.tile([P, 1], f32, tag="sums")
                nc.vector.reduce_sum(out=sums, in_=exp_cols,
                                     axis=mybir.AxisListType.X)
                den_ps = psmall.tile([1, 1], f32, tag="small")
                nc.tensor.matmul(den_ps, lhsT=sums, rhs=ones_col,
                                 start=True, stop=True)
                recip = batch_pool.tile([1, 1], f32, tag="recip")
                nc.vector.reciprocal(out=recip, in_=den_ps)
                # broadcast 1/denom to 128 partitions via K=1 matmul
                rb_ps = psmall.tile([P, 1], f32, tag="small")
                nc.tensor.matmul(rb_ps, lhsT=ones_row32, rhs=recip,
                                 start=True, stop=True)
                recip_bc = batch_pool.tile([P, 1], f32, tag="recip_bc")
                nc.vector.tensor_copy(out=recip_bc, in_=rb_ps)
                # normalized weights, transposed for contiguous store
                w_cols = batch_pool.tile([P, SB], f32, tag="w_cols")
                nc.vector.tensor_scalar_mul(w_cols, exp_cols, recip_bc)
                wt_ps = psmall.tile([SB, P], f32, tag="small")
                nc.tensor.transpose(wt_ps, w_cols, ident)
                wt_sb = out_pool.tile([SB, P], f32, tag="wt")
                nc.vector.tensor_copy(out=wt_sb, in_=wt_ps)
                nc.sync.dma_start(
                    out=wts_out[b].rearrange("(t p) -> t p", p=P), in_=wt_sb)
                # context, normalized during PSUM evacuation
                ctx_sb = out_pool.tile([1, H], f32, tag="ctxrow")
                if ctx_dve:
                    accT = st["acc"]
                    for oh in range(NH):
                        cr_ps = psmall.tile([1, NB], f32, tag="small",
                                            name=f"cr_ps_{b}_{oh}")
                        nc.tensor.matmul(
                            cr_ps,
                            lhsT=ones_col,
                            rhs=accT[:, oh * NB:(oh + 1) * NB],
                            start=True, stop=True)
                        nc.scalar.activation(out=ctx_sb[:, oh * NB:(oh + 1) * NB],
                                             in_=cr_ps, func=Act.Copy,
                                             bias=0.0, scale=recip)
                else:
                    nc.scalar.activation(out=ctx_sb, in_=st["ctx_ps"],
                                         func=Act.Copy, bias=0.0, scale=recip)
                nc.sync.dma_start(out=ctx_out[b:b + 1, :], in_=ctx_sb)
            return emit

        def emit_dec_bcast(b):
            # per-batch dec_proj broadcast [128, H] (PE K=1 matmul into a
            # psum slot, evacuated by DVE)
            db_ps = pmm.tile([P, H], f32, tag="mm", name=f"db_ps_{b}")
            for oh in range(NH):
                nc.tensor.matmul(
                    db_ps[:, oh * NB:(oh + 1) * NB],
                    lhsT=ones_row,
                    rhs=dec_rows[:, b, oh * NB:(oh + 1) * NB],
                    start=True, stop=True)
            dec_bc = gather_pool.tile([P, H], f32, tag="dec_bc",
                                      name=f"dec_bc_{b}")
            nc.vector.tensor_copy(out=dec_bc, in_=db_ps)
            state[b]["dec_bc"] = dec_bc

        emit_dec_preamble()

        G = b_loc * SB
        for g in range(G + LAG + 2):
            if g < G:
                b, sb = divmod(g, SB)
                if sb == 0:
                    state[b] = dict(
                        exp_cols=gather_pool.tile([P, SB], f32, tag="exp_cols",
                                                  name=f"exp_cols_b{b}"),
                        enc_tiles={},
                        exp_tiles={},
                        acc=None,
                    )
                    if not ctx_dve:
                        state[b]["ctx_ps"] = pctx.tile([1, H], f32, tag="ctx",
                                                       name=f"ctx_ps_b{b}")
                    if not dec_pe:
                        emit_dec_bcast(b)
                st = state[b]

                if g in prefetched:
                    enc_t_t, enc_nat = prefetched.pop(g)
                else:
                    enc_t_t = enc_t_pool.tile([P, KT * P], mm_dt, tag="enct",
                                              name=f"enct_{g}")
                    nc.sync.dma_start(out=enc_t_t, in_=enc_t[b, sb])
                    enc_nat = enc_pool.tile([P, H], mm_dt, tag="encn",
                                            name=f"encn_{g}")
                    nc.sync.dma_start(out=enc_nat,
                                      in_=enc[b, sb * P:(sb + 1) * P, :])
                st["enc_tiles"][sb] = enc_nat

                mm_ps = pmm.tile([P, H], f32, tag="mm", name=f"mm_ps_{g}")
                if dec_pe:
                    # open both bank groups with the dec_proj broadcast-add
                    for oh in range(NH):
                        nc.tensor.matmul(
                            mm_ps[:, oh * NB:(oh + 1) * NB],
                            lhsT=ones_row,
                            rhs=dec_rows[:, b, oh * NB:(oh + 1) * NB],
                            start=True, stop=False)
                # k-outer so each stationary enc_t tile serves both halves
                for kt in range(KT):
                    lhs = enc_t_t[:, kt * P:(kt + 1) * P]
                    for oh in range(NH):
                        nc.tensor.matmul(
                            mm_ps[:, oh * NB:(oh + 1) * NB],
                            lhsT=lhs,
                            rhs=w_ht_sb[:, kt * H + oh * NB: kt * H + oh * NB + NB],
                            start=(kt == 0 and not dec_pe),
                            stop=(kt == KT - 1),
                        )

                if dec_pe:
                    tanh_in = mm_ps
                else:
                    # dec_proj add on DVE (PSUM read), tanh on ACT
                    esum = scr_pool.tile([P, H], f32, tag="esum",
                                         name=f"esum_{g}")
                    nc.vector.tensor_add(esum, mm_ps, st["dec_bc"])
                    tanh_in = esum
                en_dt = mm_dt if not wide else f32
                energy = en_pool.tile([P, H], en_dt, tag="energy",
                                      name=f"energy_{g}")
                nc.scalar.activation(out=energy, in_=tanh_in, func=Act.Tanh,
                                     bias=zeros_col, scale=1.0)

                scr = scr_pool.tile([P, H], en_dt, tag="scr", name=f"scr_{g}")
                score_col = col_pool.tile([P, 1], f32, tag="score",
                                          name=f"score_{g}")
                nc.vector.scalar_tensor_tensor(
                    out=scr, in0=energy, scalar=1.0, in1=v_sb,
                    op0=Alu.mult, op1=Alu.mult, accum_out=score_col)

                exp_c = col_pool.tile([P, 1], f32, tag="expc", name=f"expc_{g}")
                nc.scalar.activation(out=exp_c, in_=score_col, func=Act.Exp,
                                     bias=zeros_col, scale=1.0)
                if ctx_dve:
                    st.setdefault("exp32_tiles", {})[sb] = exp_c
                else:
                    exp_r = col_pool.tile([P, 1], mm_dt, tag="expr",
                                          name=f"expr_{g}")
                    nc.vector.tensor_copy(out=exp_r, in_=exp_c)
                    st["exp_tiles"][sb] = exp_r
                # gather for the batch-end softmax denominator / store
                nc.vector.tensor_copy(out=st["exp_cols"][:, sb:sb + 1], in_=exp_c)

                # tighter lag for the final batch's last blocks: the tail
                # chain is fully serial, so emit dependents ASAP there
                lag = 1 if (b == b_loc - 1 and sb >= SB - 2) else LAG
                defer(g + lag, make_ctx_emitter(b, sb, st))
                if sb == SB - 1:
                    defer(g + lag, make_epilogue(b, st))

            for fn in deferred.pop(g, []):
                fn()

        assert not deferred

    nc.compile()
    return nc


def _np_mm_dtype(mm):
    if mm == "bf16":
        import ml_dtypes
        return np.dtype(ml_dtypes.bfloat16)
    if mm == "f16":
        return np.dtype(np.float16)
    return np.dtype(np.float32)


def _prep_core_inputs(dec_hidden, enc_outputs, w_ht_host, w_st_host, v_host,
                      c, b_loc, s=S, h=H, mm=MM_MODE):
    np_dt = _np_mm_dtype(mm)
    sb_, kt_ = s // P, h // P
    lo, hi = c * b_loc, (c + 1) * b_loc
    enc = np.ascontiguousarray(enc_outputs[lo:hi], dtype=np_dt)
    # [b, sb, p, kt*128] with enc_t[b,sb,p,kt*128+sw] = enc[b, sb*128+sw, kt*128+p]
    e = enc.reshape(b_loc, sb_, P, kt_, P)
    enc_t = np.ascontiguousarray(e.transpose(0, 1, 4, 3, 2)).reshape(
        b_loc, sb_, P, kt_ * P)
    dec = np.asarray(dec_hidden[lo:hi, 0, :], dtype=np_dt)  # [b_loc, h]
    dec_t = np.ascontiguousarray(
        dec.T.reshape(kt_, P, b_loc).transpose(1, 0, 2)).reshape(P, kt_ * b_loc)
    return {
        "enc_t": enc_t,
        "enc": enc,
        "w_ht": w_ht_host,
        "w_st": w_st_host,
        "dec_t": dec_t,
        "v_in": v_host,
        "ones_in": np.ones((1, P), dtype=np_dt),
        "ident_in": np.eye(P, dtype=np.float32),
    }


def _swizzle_weight(w, h=H, mm=MM_MODE):
    # [o, h] -> [p, kt*h] with w_sw[p, kt*h + o] = w[o, kt*128+p]
    kt_ = h // P
    wt = np.asarray(w).astype(_np_mm_dtype(mm)).T  # [h, o]
    return np.ascontiguousarray(
        wt.reshape(kt_, P, w.shape[0]).transpose(1, 0, 2)).reshape(P, kt_ * w.shape[0])


def kernel(dec_hidden, enc_outputs, W_h, W_s, v, _trace=False):
    from concourse.bass_utils import run_bass_kernel_spmd

    assert enc_outputs.shape == (B, S, H)
    key = ("full", MM_MODE)
    if key not in _BUILT:
        _BUILT[key] = _build(B_LOC, mm=MM_MODE)
    nc = _BUILT[key]

    w_ht_host = _swizzle_weight(W_h)
    w_st_host = _swizzle_weight(W_s)
    v_host = np.ascontiguousarray(np.asarray(v, dtype=np.float32))

    in_maps = [
        _prep_core_inputs(dec_hidden, enc_outputs, w_ht_host, w_st_host,
                          v_host, c, B_LOC)
        for c in range(NCORES)
    ]
    res = run_bass_kernel_spmd(nc, in_maps, core_ids=list(range(NCORES)),
                               trace=_trace)
    kernel.last_results = res
    ctx = np.concatenate([r["ctx_out"] for r in res.results], axis=0)
    wts = np.concatenate([r["wts_out"] for r in res.results], axis=0)
    return (ctx.reshape(B, 1, H).astype(np.float32),
            wts.astype(np.float32))
